# revision 3
# baseline (speedup 1.0000x reference)
"""k-means++ diverse sampler on 8 TRN2 NeuronCores (pure data parallel: one
batch row per core).

Device algorithm per core (row): precompute the token Gram matrix
G = X X^T (fp16 hi/lo 3-term for f32-grade accuracy) into DRAM at full PE
utilization, then run the 255-step sequential k-means++ scan on-chip:
scores s = d^2 * exp(2*gumbel) (argmax-equivalent to log d + gumbel),
DVE max8/max_index + PE transpose for the cross-partition argmax, the winner
index is loaded into engine registers, one 16KB G-row DMA provides the dot
products for the distance update.  The host precomputes all RNG (first
centers + Gumbel noise via CPU jax, matching jax.random.categorical
internals), squared norms, and the step-0 distances; the device returns the
255 sampled indices per row and the host gathers the exact f32 tokens.
"""

import sys

sys.path.insert(0, "/opt/trn_rl_repo")

import numpy as np

B, N, D, K = 8, 4096, 1024, 256
P = 128
NF = N // P
DC = D // P
JG = 512
NJ = N // JG
NSTEP = K - 1
UNROLL = 5

_compiled = {}


def _build():
    import concourse.bass as bass
    import concourse.bacc as bacc
    import concourse.mybir as mybir
    import concourse.tile as tile

    F32 = mybir.dt.float32
    F16 = mybir.dt.float16
    U32 = mybir.dt.uint32
    U16 = mybir.dt.uint16
    ALU = mybir.AluOpType
    ET = mybir.EngineType

    nc = bacc.Bacc("TRN2", target_bir_lowering=False, debug=False,
                   num_devices=8)

    ht_d = nc.dram_tensor("ht", [P, DC * N], F16, kind="ExternalInput")
    lt_d = nc.dram_tensor("lt", [P, DC * N], F16, kind="ExternalInput")
    rsf_d = nc.dram_tensor("rsf", [1, N], F32, kind="ExternalInput")
    rstm_d = nc.dram_tensor("rstm", [P, NF], F32, kind="ExternalInput")
    ee_d = nc.dram_tensor("ee", [P, NSTEP * NF], F32, kind="ExternalInput")
    d0_d = nc.dram_tensor("d0sq", [P, NF], F32, kind="ExternalInput")
    idinv_d = nc.dram_tensor("idinv", [P, P], F32, kind="ExternalInput")
    idn_d = nc.dram_tensor("idn", [P, P], F32, kind="ExternalInput")
    iota_d = nc.dram_tensor("iotap", [P, 1], F32, kind="ExternalInput")
    g_d = nc.dram_tensor("gram", [N, N], F32, kind="Internal")
    idx_d = nc.dram_tensor("idx_out", [1, NSTEP], U32, kind="ExternalOutput")

    g3 = g_d.ap().rearrange("a (p f) -> a p f", p=P)

    with tile.TileContext(nc) as tc:
        with (
            tc.tile_pool(name="persist", bufs=1) as pp,
            tc.tile_pool(name="gout", bufs=3) as gp_pool,
            tc.tile_pool(name="psum", bufs=1, space=bass.MemorySpace.PSUM) as psp,
            tc.tile_pool(name="psum2", bufs=2, space=bass.MemorySpace.PSUM) as psp2,
            tc.tile_pool(name="scan", bufs=2) as sp,
        ):
            ee_sb = pp.tile([P, NSTEP * NF], F32)
            rstm_sb = pp.tile([P, NF], F32)
            rsf_sb = pp.tile([1, N], F32)
            d2 = pp.tile([P, NF], F32)
            idinv_sb = pp.tile([P, P], F32)
            idn_sb = pp.tile([P, P], F32)
            iota_sb = pp.tile([P, 1], F32)
            rs_rep = pp.tile([P, N], F32)
            idxbuf = pp.tile([1, NSTEP], U32)
            ones_sb = pp.tile([1, P], F32)

            nc.sync.dma_start(ee_sb, ee_d.ap())
            nc.sync.dma_start(rstm_sb, rstm_d.ap())
            nc.sync.dma_start(rsf_sb, rsf_d.ap())
            nc.sync.dma_start(d2, d0_d.ap())
            nc.sync.dma_start(idinv_sb, idinv_d.ap())
            nc.sync.dma_start(idn_sb, idn_d.ap())
            nc.sync.dma_start(iota_sb, iota_d.ap())
            nc.vector.memset(ones_sb, 1.0)

            for jg in range(NJ):
                ps = psp.tile([P, JG], F32, tag=f"g{jg % 4}", name=f"bc{jg}")
                nc.tensor.matmul(ps, ones_sb, rsf_sb[0:1, jg * JG:(jg + 1) * JG],
                                 start=True, stop=True)
                nc.any.tensor_copy(rs_rep[:, jg * JG:(jg + 1) * JG], ps)

            ht_sb = pp.tile([P, DC * N], F16)
            lt_sb = pp.tile([P, DC * N], F16)
            nc.sync.dma_start(ht_sb, ht_d.ap())
            nc.sync.dma_start(lt_sb, lt_d.ap())

            njh = max(NJ // 2, 1)
            for it in range(N // P):
                for jh in range((NJ + njh - 1) // njh):
                    jgs = range(jh * njh, min((jh + 1) * njh, NJ))
                    psg = {jg: psp.tile([P, JG], F32, tag=f"g{jg % 4}",
                                        name=f"psg{jg}")
                           for jg in jgs}
                    for c in range(DC):
                        wh = ht_sb[:, c * N + it * P: c * N + (it + 1) * P]
                        wl = lt_sb[:, c * N + it * P: c * N + (it + 1) * P]
                        for jg in jgs:
                            rh = ht_sb[:, c * N + jg * JG: c * N + jg * JG + JG]
                            rl = lt_sb[:, c * N + jg * JG: c * N + jg * JG + JG]
                            nc.tensor.matmul(psg[jg], wh, rh, start=(c == 0),
                                             stop=False)
                            nc.tensor.matmul(psg[jg], wh, rl, start=False,
                                             stop=False)
                            nc.tensor.matmul(psg[jg], wl, rh, start=False,
                                             stop=(c == DC - 1))
                    for jg in jgs:
                        gbuf = gp_pool.tile([P, JG], F32, tag="gbuf",
                                            name=f"gb{jg}")
                        nc.any.tensor_copy(gbuf, psg[jg])
                        nc.sync.dma_start(
                            g_d.ap()[it * P:(it + 1) * P,
                                     jg * JG:(jg + 1) * JG],
                            gbuf)

            def body(k):
                s = sp.tile([P, NF], F32, tag="s", name="s")
                pm = sp.tile([P, 8], F32, tag="pm", name="pm")
                pi = sp.tile([P, 8], U16, tag="pi", name="pi")
                pif = sp.tile([P, 1], F32, tag="pif", name="pif")
                fidx = sp.tile([P, 1], F32, tag="fidx", name="fidx")
                rowv = sp.tile([1, P], F32, tag="rowv", name="rowv")
                row1u = sp.tile([1, P], U32, tag="row1u", name="row1u")
                gm = sp.tile([1, 8], F32, tag="gm", name="gm")
                gpos = sp.tile([1, 8], U32, tag="gpos", name="gpos")
                gcol = sp.tile([P, NF], F32, tag="gcol", name="gcol")
                rc_bc = sp.tile([P, 1], F32, tag="rc", name="rc_bc")
                invcol = sp.tile([P, 1], F32, tag="inv", name="invcol")
                t1 = sp.tile([P, NF], F32, tag="t1", name="t1")
                nd2 = sp.tile([P, NF], F32, tag="nd2", name="nd2")
                tpv = psp2.tile([1, P], F32, tag="tpv", name="tpv")
                tpi = psp2.tile([1, P], F32, tag="tpi", name="tpi")

                nc.vector.tensor_tensor(s, d2, ee_sb[:, bass.ds(k * NF, NF)],
                                        ALU.mult)
                nc.vector.max(pm, s)
                nc.vector.max_index(pi, pm, s)
                nc.vector.tensor_copy(pif, pi[:, 0:1])
                nc.vector.tensor_scalar(fidx, pif, iota_sb, None, ALU.add)
                nc.tensor.transpose(tpv, pm[:, 0:1], idn_sb)
                nc.tensor.transpose(tpi, fidx, idn_sb)
                nc.vector.tensor_copy(rowv, tpv)
                nc.vector.tensor_copy(row1u, tpi)
                nc.vector.max(gm, rowv)
                nc.vector.max_index(gpos, gm, rowv)

                engines = bass.OrderedSet([ET.SP, ET.Activation, ET.DVE])
                r_p = nc.values_load(gpos[0:1, 0:1], engines=engines,
                                     min_val=0, max_val=P - 1,
                                     skip_runtime_bounds_check=True)
                r_flat = nc.values_load(row1u[0:1, bass.ds(r_p, 1)],
                                        engines=engines, min_val=0,
                                        max_val=N - 1,
                                        skip_runtime_bounds_check=True)

                nc.sync.reg_save(idxbuf[0:1, bass.ds(k, 1)], r_flat)
                nc.sync.dma_start(gcol, g3[bass.ds(r_flat, 1), :, :])
                nc.scalar.copy(rc_bc, rs_rep[:, bass.ds(r_flat, 1)])
                nc.scalar.copy(invcol, idinv_sb[:, bass.ds(r_flat // NF, 1)])

                nc.vector.tensor_scalar(t1, gcol, -2.0, rc_bc, ALU.mult,
                                        ALU.add)
                nc.vector.tensor_tensor(nd2, t1, rstm_sb, ALU.add)
                nc.vector.tensor_tensor(d2, d2, nd2, ALU.min)
                dcol = d2[:, bass.ds(r_flat % NF, 1)]
                nc.vector.tensor_tensor(dcol, dcol, invcol, ALU.mult)

            tc.For_i_unrolled(0, NSTEP, 1, body, max_unroll=UNROLL)

            nc.sync.dma_start(idx_d.ap(), idxbuf)

    nc.compile()
    return nc


def _host_rng():
    """first indices + gumbel noise, matching the reference's vmap/scan
    structure bit-exactly (CPU jax)."""
    import jax
    import jax.numpy as jnp

    cpu = jax.devices("cpu")[0]
    with jax.default_device(cpu):
        keys = jax.random.split(jax.random.key(42), B)

        def rng_row(key):
            k_first, k_rest = jax.random.split(key)
            first = jax.random.randint(k_first, (), 0, N)

            def step(c, kk):
                return c, jax.random.gumbel(kk, (N,), jnp.float32)

            _, gs = jax.lax.scan(step, 0.0, jax.random.split(k_rest, K - 1))
            return first, gs

        firsts, gs = jax.vmap(rng_row)(keys)
        return np.asarray(jax.device_get(firsts)), np.asarray(
            jax.device_get(gs))


def _make_inputs_row(x_row, first, g_row):
    xf = np.ascontiguousarray(x_row, dtype=np.float32)
    H = xf.astype(np.float16)
    L = (xf - H.astype(np.float32)).astype(np.float16)

    def to_t(a):  # [N, D] -> [P, DC*N] : out[p, c*N + t] = a[t, c*P + p]
        t = a.T.reshape(DC, P, N)
        return np.ascontiguousarray(t.transpose(1, 0, 2).reshape(P, DC * N))

    x64 = xf.astype(np.float64)
    rs64 = (x64 ** 2).sum(-1)
    rsf = rs64.astype(np.float32)[None, :]
    rstm = rs64.astype(np.float32).reshape(P, NF)
    d0 = ((x64 - x64[first]) ** 2).sum(-1)
    d0[first] = 0.0
    d0 = np.maximum(d0, 0.0).astype(np.float32).reshape(P, NF)
    ee = np.exp(2.0 * g_row.astype(np.float64)).astype(np.float32)
    ee_sb = np.ascontiguousarray(
        ee.reshape(NSTEP, P, NF).transpose(1, 0, 2).reshape(P, NSTEP * NF))
    idinv = (1.0 - np.eye(P)).astype(np.float32)
    idn = np.eye(P, dtype=np.float32)
    iota = (np.arange(P, dtype=np.float32) * NF)[:, None]
    return {
        "ht": to_t(H), "lt": to_t(L), "rsf": rsf, "rstm": rstm,
        "ee": ee_sb, "d0sq": d0, "idinv": idinv, "idn": idn,
        "iotap": np.ascontiguousarray(iota),
    }


def kernel(x):
    from concourse.bass_utils import run_bass_kernel_spmd
    import jax

    x_np = np.asarray(jax.device_get(x), dtype=np.float32)
    assert x_np.shape == (B, N, D), x_np.shape

    firsts, gs = _host_rng()

    if "nc" not in _compiled:
        _compiled["nc"] = _build()
    nc = _compiled["nc"]

    in_maps = [_make_inputs_row(x_np[b], int(firsts[b]), gs[b])
               for b in range(B)]
    res = run_bass_kernel_spmd(nc, in_maps, core_ids=list(range(B)))

    tokens = np.empty((B, K, D), dtype=np.float32)
    for b in range(B):
        idx = np.concatenate(
            [[int(firsts[b])],
             res.results[b]["idx_out"].ravel().astype(np.int64)])
        tokens[b] = x_np[b][idx]
    return (tokens, 0.0)


# revision 4
# speedup vs baseline: 1.5587x; 1.5587x over previous
"""k-means++ diverse sampler on 8 TRN2 NeuronCores (pure data parallel: one
batch row per core).

Device algorithm per core (row): precompute G' = rs_i - 2 * X X^T row-wise
(fp16 hi/lo 3-term matmuls for f32-grade dot products) into DRAM at full PE
utilization, then run the 255-step sequential k-means++ scan on-chip:
scores s = d^2 * exp(2*gumbel) (argmax-equivalent to the reference's
log d + gumbel), DVE max8/max_index + PE transpose for the cross-partition
argmax, the winner index is loaded into engine registers, and one 16KB
G'-row DMA provides the distance update nd^2 = G'row + rs.  The host
precomputes all RNG (first centers + Gumbel noise via CPU jax, bit-matching
jax.random.categorical under the reference's vmap/scan structure), squared
norms, and step-0 distances; the device returns the 255 sampled indices per
row and the host gathers the exact f32 tokens.
"""

import sys

sys.path.insert(0, "/opt/trn_rl_repo")

import numpy as np

B, N, D, K = 8, 4096, 1024, 256
P = 128
NF = N // P
DC = D // P
JG = 512
NJ = N // JG
NSTEP = K - 1
UNROLL = 5

_compiled = {}


def _build():
    import concourse.bass as bass
    import concourse.bacc as bacc
    import concourse.mybir as mybir
    import concourse.tile as tile

    F32 = mybir.dt.float32
    F16 = mybir.dt.float16
    U32 = mybir.dt.uint32
    U16 = mybir.dt.uint16
    AF = mybir.ActivationFunctionType
    ALU = mybir.AluOpType
    ET = mybir.EngineType

    nc = bacc.Bacc("TRN2", target_bir_lowering=False, debug=False,
                   num_devices=8)

    ht_d = nc.dram_tensor("ht", [P, DC * N], F16, kind="ExternalInput")
    lt_d = nc.dram_tensor("lt", [P, DC * N], F16, kind="ExternalInput")
    rspm_d = nc.dram_tensor("rspm", [P, N // P], F32, kind="ExternalInput")
    rstm_d = nc.dram_tensor("rstm", [P, NF], F32, kind="ExternalInput")
    ee_d = nc.dram_tensor("ee", [P, NSTEP * NF], F32, kind="ExternalInput")
    d0_d = nc.dram_tensor("d0sq", [P, NF], F32, kind="ExternalInput")
    idinv_d = nc.dram_tensor("idinv", [P, P], F32, kind="ExternalInput")
    idn_d = nc.dram_tensor("idn", [P, P], F32, kind="ExternalInput")
    iota_d = nc.dram_tensor("iotap", [P, 1], F32, kind="ExternalInput")
    g_d = nc.dram_tensor("gram", [N, N], F32, kind="Internal")
    idx_d = nc.dram_tensor("idx_out", [1, NSTEP], U32, kind="ExternalOutput")

    g3 = g_d.ap().rearrange("a (p f) -> a p f", p=P)

    with tile.TileContext(nc) as tc:
        with (
            tc.tile_pool(name="persist", bufs=1) as pp,
            tc.tile_pool(name="gout", bufs=3) as gp_pool,
            tc.tile_pool(name="psum", bufs=1, space=bass.MemorySpace.PSUM) as psp,
            tc.tile_pool(name="psum2", bufs=2, space=bass.MemorySpace.PSUM) as psp2,
            tc.tile_pool(name="scan", bufs=2) as sp,
        ):
            ee_sb = pp.tile([P, NSTEP * NF], F32)
            rstm_sb = pp.tile([P, NF], F32)
            rspm_sb = pp.tile([P, N // P], F32)
            d2 = pp.tile([P, NF], F32)
            idinv_sb = pp.tile([P, P], F32)
            idn_sb = pp.tile([P, P], F32)
            iota_sb = pp.tile([P, 1], F32)
            idxbuf = pp.tile([1, NSTEP], U32)

            nc.sync.dma_start(ee_sb, ee_d.ap())
            nc.sync.dma_start(rstm_sb, rstm_d.ap())
            nc.sync.dma_start(rspm_sb, rspm_d.ap())
            nc.sync.dma_start(d2, d0_d.ap())
            nc.sync.dma_start(idinv_sb, idinv_d.ap())
            nc.sync.dma_start(idn_sb, idn_d.ap())
            nc.sync.dma_start(iota_sb, iota_d.ap())

            ht_sb = pp.tile([P, DC * N], F16)
            lt_sb = pp.tile([P, DC * N], F16)
            nc.sync.dma_start(ht_sb, ht_d.ap())
            nc.sync.dma_start(lt_sb, lt_d.ap())

            njh = max(NJ // 2, 1)
            for it in range(N // P):
                for jh in range((NJ + njh - 1) // njh):
                    jgs = range(jh * njh, min((jh + 1) * njh, NJ))
                    psg = {jg: psp.tile([P, JG], F32, tag=f"g{jg % 4}",
                                        name=f"psg{jg}")
                           for jg in jgs}
                    for c in range(DC):
                        wh = ht_sb[:, c * N + it * P: c * N + (it + 1) * P]
                        wl = lt_sb[:, c * N + it * P: c * N + (it + 1) * P]
                        for jg in jgs:
                            rh = ht_sb[:, c * N + jg * JG: c * N + jg * JG + JG]
                            rl = lt_sb[:, c * N + jg * JG: c * N + jg * JG + JG]
                            nc.tensor.matmul(psg[jg], wh, rh, start=(c == 0),
                                             stop=False)
                            nc.tensor.matmul(psg[jg], wh, rl, start=False,
                                             stop=False)
                            nc.tensor.matmul(psg[jg], wl, rh, start=False,
                                             stop=(c == DC - 1))
                    for jg in jgs:
                        gbuf = gp_pool.tile([P, JG], F32, tag="gbuf",
                                            name=f"gb{jg}")
                        if jg % 2 == 0:
                            nc.vector.tensor_scalar(
                                gbuf, psg[jg], -2.0,
                                rspm_sb[:, it:it + 1], ALU.mult, ALU.add)
                        else:
                            nc.scalar.activation(
                                gbuf, psg[jg], AF.Identity,
                                bias=rspm_sb[:, it:it + 1], scale=-2.0)
                        nc.sync.dma_start(
                            g_d.ap()[it * P:(it + 1) * P,
                                     jg * JG:(jg + 1) * JG],
                            gbuf)

            def body(k):
                s = sp.tile([P, NF], F32, tag="s", name="s")
                pm = sp.tile([P, 8], F32, tag="pm", name="pm")
                pi = sp.tile([P, 8], U16, tag="pi", name="pi")
                fidx = sp.tile([P, 1], F32, tag="fidx", name="fidx")
                rowv = sp.tile([1, P], F32, tag="rowv", name="rowv")
                row1u = sp.tile([1, P], U32, tag="row1u", name="row1u")
                gm = sp.tile([1, 8], F32, tag="gm", name="gm")
                gpos = sp.tile([1, 8], U32, tag="gpos", name="gpos")
                gcol = sp.tile([P, NF], F32, tag="gcol", name="gcol")
                invcol = sp.tile([P, 1], F32, tag="inv", name="invcol")
                nd2 = sp.tile([P, NF], F32, tag="nd2", name="nd2")
                tpv = psp2.tile([1, P], F32, tag="tpv", name="tpv")
                tpi = psp2.tile([1, P], F32, tag="tpi", name="tpi")

                nc.vector.tensor_tensor(s, d2, ee_sb[:, bass.ds(k * NF, NF)],
                                        ALU.mult)
                nc.vector.max(pm, s)
                nc.vector.max_index(pi, pm, s)
                nc.vector.tensor_scalar(fidx, pi[:, 0:1], iota_sb, None,
                                        ALU.add)
                nc.tensor.transpose(tpv, pm[:, 0:1], idn_sb)
                nc.tensor.transpose(tpi, fidx, idn_sb)
                nc.vector.tensor_copy(rowv, tpv)
                nc.scalar.copy(row1u, tpi)
                nc.vector.max(gm, rowv)
                nc.vector.max_index(gpos, gm, rowv)

                engines = bass.OrderedSet([ET.SP, ET.Activation, ET.DVE])
                r_p = nc.values_load(gpos[0:1, 0:1], engines=engines,
                                     min_val=0, max_val=P - 1,
                                     skip_runtime_bounds_check=True)
                r_flat = nc.values_load(row1u[0:1, bass.ds(r_p, 1)],
                                        engines=engines, min_val=0,
                                        max_val=N - 1,
                                        skip_runtime_bounds_check=True)

                nc.sync.reg_save(idxbuf[0:1, bass.ds(k, 1)], r_flat)
                nc.sync.dma_start(gcol, g3[bass.ds(r_flat, 1), :, :])
                nc.scalar.copy(invcol, idinv_sb[:, bass.ds(r_flat // NF, 1)])

                nc.vector.tensor_tensor(nd2, gcol, rstm_sb, ALU.add)
                nc.vector.tensor_tensor(d2, d2, nd2, ALU.min)
                dcol = d2[:, bass.ds(r_flat % NF, 1)]
                nc.vector.tensor_tensor(dcol, dcol, invcol, ALU.mult)

            tc.For_i_unrolled(0, NSTEP, 1, body, max_unroll=UNROLL)

            nc.sync.dma_start(idx_d.ap(), idxbuf)

    nc.compile()
    return nc


def _host_rng():
    """First indices + Gumbel noise, matching the reference's vmap/scan
    structure bit-exactly (CPU jax)."""
    import jax
    import jax.numpy as jnp

    cpu = jax.devices("cpu")[0]
    with jax.default_device(cpu):
        keys = jax.random.split(jax.random.key(42), B)

        def rng_row(key):
            k_first, k_rest = jax.random.split(key)
            first = jax.random.randint(k_first, (), 0, N)

            def step(c, kk):
                return c, jax.random.gumbel(kk, (N,), jnp.float32)

            _, gs = jax.lax.scan(step, 0.0, jax.random.split(k_rest, K - 1))
            return first, gs

        firsts, gs = jax.vmap(rng_row)(keys)
        return np.asarray(jax.device_get(firsts)), np.asarray(
            jax.device_get(gs))


def _make_inputs_row(x_row, first, g_row):
    xf = np.ascontiguousarray(x_row, dtype=np.float32)
    H = xf.astype(np.float16)
    L = (xf - H.astype(np.float32)).astype(np.float16)

    def to_t(a):  # [N, D] -> [P, DC*N] : out[p, c*N + t] = a[t, c*P + p]
        t = a.T.reshape(DC, P, N)
        return np.ascontiguousarray(t.transpose(1, 0, 2).reshape(P, DC * N))

    x64 = xf.astype(np.float64)
    rs64 = (x64 ** 2).sum(-1)
    rs32 = rs64.astype(np.float32)
    rstm = rs32.reshape(P, NF)                                  # pmajor
    rspm = np.ascontiguousarray(rs32.reshape(N // P, P).T)      # pminor
    d0 = ((x64 - x64[first]) ** 2).sum(-1)
    d0[first] = 0.0
    d0 = np.maximum(d0, 0.0).astype(np.float32).reshape(P, NF)
    ee = np.exp(2.0 * g_row.astype(np.float64)).astype(np.float32)
    ee_sb = np.ascontiguousarray(
        ee.reshape(NSTEP, P, NF).transpose(1, 0, 2).reshape(P, NSTEP * NF))
    idinv = (1.0 - np.eye(P)).astype(np.float32)
    idn = np.eye(P, dtype=np.float32)
    iota = (np.arange(P, dtype=np.float32) * NF)[:, None]
    return {
        "ht": to_t(H), "lt": to_t(L), "rspm": rspm, "rstm": rstm,
        "ee": ee_sb, "d0sq": d0, "idinv": idinv, "idn": idn,
        "iotap": np.ascontiguousarray(iota),
    }


def kernel(x):
    from concourse.bass_utils import run_bass_kernel_spmd
    import jax

    x_np = np.asarray(jax.device_get(x), dtype=np.float32)
    assert x_np.shape == (B, N, D), x_np.shape

    firsts, gs = _host_rng()

    if "nc" not in _compiled:
        _compiled["nc"] = _build()
    nc = _compiled["nc"]

    in_maps = [_make_inputs_row(x_np[b], int(firsts[b]), gs[b])
               for b in range(B)]
    res = run_bass_kernel_spmd(nc, in_maps, core_ids=list(range(B)))

    tokens = np.empty((B, K, D), dtype=np.float32)
    for b in range(B):
        idx = np.concatenate(
            [[int(firsts[b])],
             res.results[b]["idx_out"].ravel().astype(np.int64)])
        tokens[b] = x_np[b][idx]
    return (tokens, 0.0)


# revision 5
# speedup vs baseline: 1.7382x; 1.1151x over previous
"""k-means++ diverse sampler on 8 TRN2 NeuronCores (pure data parallel: one
batch row per core).

Device algorithm per core (row): precompute G' = rs_i - 2 * X X^T row-wise
(fp16 hi/lo 3-term matmuls, upper-triangle tiles only + transpose-mirroring
for the symmetric lower half) into DRAM at full PE
utilization, then run the 255-step sequential k-means++ scan on-chip:
scores s = d^2 * exp(2*gumbel) (argmax-equivalent to the reference's
log d + gumbel), DVE max8/max_index + PE transpose for the cross-partition
argmax, the winner index is loaded into engine registers, and one 16KB
G'-row DMA provides the distance update nd^2 = G'row + rs.  The host
precomputes all RNG (first centers + Gumbel noise via CPU jax, bit-matching
jax.random.categorical under the reference's vmap/scan structure), squared
norms, and step-0 distances; the device returns the 255 sampled indices per
row and the host gathers the exact f32 tokens.
"""

import sys

sys.path.insert(0, "/opt/trn_rl_repo")

import numpy as np

B, N, D, K = 8, 4096, 1024, 256
P = 128
NF = N // P
DC = D // P
JG = 512
NJ = N // JG
NSTEP = K - 1
UNROLL = 5

_compiled = {}


def _build():
    import concourse.bass as bass
    import concourse.bacc as bacc
    import concourse.mybir as mybir
    import concourse.tile as tile

    F32 = mybir.dt.float32
    F16 = mybir.dt.float16
    U32 = mybir.dt.uint32
    U16 = mybir.dt.uint16
    AF = mybir.ActivationFunctionType
    ALU = mybir.AluOpType
    ET = mybir.EngineType

    nc = bacc.Bacc("TRN2", target_bir_lowering=False, debug=False,
                   num_devices=8)

    ht_d = nc.dram_tensor("ht", [P, DC * N], F16, kind="ExternalInput")
    lt_d = nc.dram_tensor("lt", [P, DC * N], F16, kind="ExternalInput")
    rspm_d = nc.dram_tensor("rspm", [P, N // P], F32, kind="ExternalInput")
    rstm_d = nc.dram_tensor("rstm", [P, NF], F32, kind="ExternalInput")
    ee_d = nc.dram_tensor("ee", [P, NSTEP * NF], F32, kind="ExternalInput")
    d0_d = nc.dram_tensor("d0sq", [P, NF], F32, kind="ExternalInput")
    idinv_d = nc.dram_tensor("idinv", [P, P], F32, kind="ExternalInput")
    idn_d = nc.dram_tensor("idn", [P, P], F32, kind="ExternalInput")
    iota_d = nc.dram_tensor("iotap", [P, 1], F32, kind="ExternalInput")
    g_d = nc.dram_tensor("gram", [N, N], F32, kind="Internal")
    idx_d = nc.dram_tensor("idx_out", [1, NSTEP], U32, kind="ExternalOutput")

    g3 = g_d.ap().rearrange("a (p f) -> a p f", p=P)

    with tile.TileContext(nc) as tc:
        with (
            tc.tile_pool(name="persist", bufs=1) as pp,
            tc.tile_pool(name="gout", bufs=3) as gp_pool,
            tc.tile_pool(name="psum", bufs=1, space=bass.MemorySpace.PSUM) as psp,
            tc.tile_pool(name="psum2", bufs=1, space=bass.MemorySpace.PSUM) as psp2,
            tc.tile_pool(name="scan", bufs=2) as sp,
        ):
            ee_sb = pp.tile([P, NSTEP * NF], F32)
            rstm_sb = pp.tile([P, NF], F32)
            rspm_sb = pp.tile([P, N // P], F32)
            d2 = pp.tile([P, NF], F32)
            idinv_sb = pp.tile([P, P], F32)
            idn_sb = pp.tile([P, P], F32)
            iota_sb = pp.tile([P, 1], F32)
            idxbuf = pp.tile([1, NSTEP], U32)

            nc.sync.dma_start(ee_sb, ee_d.ap())
            nc.sync.dma_start(rstm_sb, rstm_d.ap())
            nc.sync.dma_start(rspm_sb, rspm_d.ap())
            nc.sync.dma_start(d2, d0_d.ap())
            nc.sync.dma_start(idinv_sb, idinv_d.ap())
            nc.sync.dma_start(idn_sb, idn_d.ap())
            nc.sync.dma_start(iota_sb, iota_d.ap())

            ht_sb = pp.tile([P, DC * N], F16)
            lt_sb = pp.tile([P, DC * N], F16)
            nc.sync.dma_start(ht_sb, ht_d.ap())
            nc.sync.dma_start(lt_sb, lt_d.ap())

            RB = JG // P
            NIT = N // P
            for it in range(NIT):
                jgs_all = [jg for jg in range(NJ) if it <= RB * jg + RB - 1]
                for h0 in range(0, len(jgs_all), 4):
                    jgs = jgs_all[h0:h0 + 4]
                    psg = {jg: psp.tile([P, JG], F32, tag=f"g{jg % 4}",
                                        name=f"psg{jg}")
                           for jg in jgs}
                    for c in range(DC):
                        wh = ht_sb[:, c * N + it * P: c * N + (it + 1) * P]
                        wl = lt_sb[:, c * N + it * P: c * N + (it + 1) * P]
                        for jg in jgs:
                            rh = ht_sb[:, c * N + jg * JG: c * N + jg * JG + JG]
                            rl = lt_sb[:, c * N + jg * JG: c * N + jg * JG + JG]
                            nc.tensor.matmul(psg[jg], wh, rh, start=(c == 0),
                                             stop=False)
                            nc.tensor.matmul(psg[jg], wh, rl, start=False,
                                             stop=False)
                            nc.tensor.matmul(psg[jg], wl, rh, start=False,
                                             stop=(c == DC - 1))
                    for jg in jgs:
                        graw = gp_pool.tile([P, JG], F32, tag="graw",
                                            name=f"gr{jg}")
                        nc.vector.tensor_copy(graw, psg[jg])
                        gbuf = gp_pool.tile([P, JG], F32, tag="gbuf",
                                            name=f"gb{jg}")
                        nc.scalar.activation(
                            gbuf, graw, AF.Identity,
                            bias=rspm_sb[:, it:it + 1], scale=-2.0)
                        nc.sync.dma_start(
                            g_d.ap()[it * P:(it + 1) * P,
                                     jg * JG:(jg + 1) * JG],
                            gbuf)
                        for q in range(RB):
                            itm = RB * jg + q
                            if itm <= RB * (it // RB) + RB - 1:
                                continue  # mirror target computed directly
                            tpm = psp2.tile([P, P], F32, tag="mir",
                                            name="tpm")
                            nc.tensor.transpose(
                                tpm, graw[:, q * P:(q + 1) * P], idn_sb)
                            mbuf = gp_pool.tile([P, P], F32, tag="mbuf",
                                                name="mbuf")
                            nc.vector.tensor_scalar(
                                mbuf, tpm, -2.0,
                                rspm_sb[:, itm:itm + 1], ALU.mult, ALU.add)
                            nc.sync.dma_start(
                                g_d.ap()[itm * P:(itm + 1) * P,
                                         it * P:(it + 1) * P],
                                mbuf)

            def body(k):
                s = sp.tile([P, NF], F32, tag="s", name="s")
                pm = sp.tile([P, 8], F32, tag="pm", name="pm")
                pi = sp.tile([P, 8], U16, tag="pi", name="pi")
                fidx = sp.tile([P, 1], F32, tag="fidx", name="fidx")
                rowv = sp.tile([1, P], F32, tag="rowv", name="rowv")
                row1u = sp.tile([1, P], U32, tag="row1u", name="row1u")
                gm = sp.tile([1, 8], F32, tag="gm", name="gm")
                gpos = sp.tile([1, 8], U32, tag="gpos", name="gpos")
                gcol = sp.tile([P, NF], F32, tag="gcol", name="gcol")
                invcol = sp.tile([P, 1], F32, tag="inv", name="invcol")
                nd2 = sp.tile([P, NF], F32, tag="nd2", name="nd2")
                tpv = psp2.tile([1, P], F32, tag="tpv", name="tpv")
                tpi = psp2.tile([1, P], F32, tag="tpi", name="tpi")

                nc.vector.tensor_tensor(s, d2, ee_sb[:, bass.ds(k * NF, NF)],
                                        ALU.mult)
                nc.vector.max(pm, s)
                nc.vector.max_index(pi, pm, s)
                nc.vector.tensor_scalar(fidx, pi[:, 0:1], iota_sb, None,
                                        ALU.add)
                nc.tensor.transpose(tpv, pm[:, 0:1], idn_sb)
                nc.tensor.transpose(tpi, fidx, idn_sb)
                nc.vector.tensor_copy(rowv, tpv)
                nc.scalar.copy(row1u, tpi)
                nc.vector.max(gm, rowv)
                nc.vector.max_index(gpos, gm, rowv)

                engines = bass.OrderedSet([ET.SP, ET.Activation, ET.DVE])
                r_p = nc.values_load(gpos[0:1, 0:1], engines=engines,
                                     min_val=0, max_val=P - 1,
                                     skip_runtime_bounds_check=True)
                r_flat = nc.values_load(row1u[0:1, bass.ds(r_p, 1)],
                                        engines=engines, min_val=0,
                                        max_val=N - 1,
                                        skip_runtime_bounds_check=True)

                nc.sync.reg_save(idxbuf[0:1, bass.ds(k, 1)], r_flat)
                nc.sync.dma_start(gcol, g3[bass.ds(r_flat, 1), :, :])
                nc.scalar.copy(invcol, idinv_sb[:, bass.ds(r_flat // NF, 1)])

                nc.vector.tensor_tensor(nd2, gcol, rstm_sb, ALU.add)
                nc.vector.tensor_tensor(d2, d2, nd2, ALU.min)
                dcol = d2[:, bass.ds(r_flat % NF, 1)]
                nc.vector.tensor_tensor(dcol, dcol, invcol, ALU.mult)

            tc.For_i_unrolled(0, NSTEP, 1, body, max_unroll=UNROLL)

            nc.sync.dma_start(idx_d.ap(), idxbuf)

    nc.compile()
    return nc


def _host_rng():
    """First indices + Gumbel noise, matching the reference's vmap/scan
    structure bit-exactly (CPU jax)."""
    import jax
    import jax.numpy as jnp

    cpu = jax.devices("cpu")[0]
    with jax.default_device(cpu):
        keys = jax.random.split(jax.random.key(42), B)

        def rng_row(key):
            k_first, k_rest = jax.random.split(key)
            first = jax.random.randint(k_first, (), 0, N)

            def step(c, kk):
                return c, jax.random.gumbel(kk, (N,), jnp.float32)

            _, gs = jax.lax.scan(step, 0.0, jax.random.split(k_rest, K - 1))
            return first, gs

        firsts, gs = jax.vmap(rng_row)(keys)
        return np.asarray(jax.device_get(firsts)), np.asarray(
            jax.device_get(gs))


def _make_inputs_row(x_row, first, g_row):
    xf = np.ascontiguousarray(x_row, dtype=np.float32)
    H = xf.astype(np.float16)
    L = (xf - H.astype(np.float32)).astype(np.float16)

    def to_t(a):  # [N, D] -> [P, DC*N] : out[p, c*N + t] = a[t, c*P + p]
        t = a.T.reshape(DC, P, N)
        return np.ascontiguousarray(t.transpose(1, 0, 2).reshape(P, DC * N))

    x64 = xf.astype(np.float64)
    rs64 = (x64 ** 2).sum(-1)
    rs32 = rs64.astype(np.float32)
    rstm = rs32.reshape(P, NF)                                  # pmajor
    rspm = np.ascontiguousarray(rs32.reshape(N // P, P).T)      # pminor
    d0 = ((x64 - x64[first]) ** 2).sum(-1)
    d0[first] = 0.0
    d0 = np.maximum(d0, 0.0).astype(np.float32).reshape(P, NF)
    ee = np.exp(2.0 * g_row.astype(np.float64)).astype(np.float32)
    ee_sb = np.ascontiguousarray(
        ee.reshape(NSTEP, P, NF).transpose(1, 0, 2).reshape(P, NSTEP * NF))
    idinv = (1.0 - np.eye(P)).astype(np.float32)
    idn = np.eye(P, dtype=np.float32)
    iota = (np.arange(P, dtype=np.float32) * NF)[:, None]
    return {
        "ht": to_t(H), "lt": to_t(L), "rspm": rspm, "rstm": rstm,
        "ee": ee_sb, "d0sq": d0, "idinv": idinv, "idn": idn,
        "iotap": np.ascontiguousarray(iota),
    }


def kernel(x):
    from concourse.bass_utils import run_bass_kernel_spmd
    import jax

    x_np = np.asarray(jax.device_get(x), dtype=np.float32)
    assert x_np.shape == (B, N, D), x_np.shape

    firsts, gs = _host_rng()

    if "nc" not in _compiled:
        _compiled["nc"] = _build()
    nc = _compiled["nc"]

    in_maps = [_make_inputs_row(x_np[b], int(firsts[b]), gs[b])
               for b in range(B)]
    res = run_bass_kernel_spmd(nc, in_maps, core_ids=list(range(B)))

    tokens = np.empty((B, K, D), dtype=np.float32)
    for b in range(B):
        idx = np.concatenate(
            [[int(firsts[b])],
             res.results[b]["idx_out"].ravel().astype(np.int64)])
        tokens[b] = x_np[b][idx]
    return (tokens, 0.0)
